# revision 9
# baseline (speedup 1.0000x reference)
"""Trainium2 Bass kernel: ArgumentRelationAttention.

out[b] = softmax_j(mask_diag(x[b] @ W @ x[b]^T + bias)) @ x[b]
  x: [64, 512, 768] f32, W: [768, 768] f32, bias: [1] f32

Strategy: pure batch data parallelism — 8 batches per NeuronCore x 8 cores.
Per batch everything stays on-chip. The PE streams ~1 col/cycle for every
dtype (measured: 227ns f32r / 216ns bf16 per 512-free matmul), so the
score path stays f32r (full precision, zero dtype conversions — DMA
writes f32r directly) and only the softmax/output stage drops to bf16:

  xT   = PE-transpose(x), f32r; each transpose group is interleaved
         between mmA accumulation groups so its PSUM evacuation (the
         transpose-phase pacer — only 2 psT banks) hides under ~1.4us of
         matmul streaming instead of back-to-back transposes.
  xWt[k,i] = sum_h W[h,k] xT[h,i]            (36 mm f32r)
  S^T  = scores TRANSPOSED: stationary xT[:, jchunk], moving xWt
         -> St[j, i] (24 mm); a 25th accumulation matmul per chunk
         (lhsT = -30000*I, rhs = one-hot slab) adds the diagonal mask
         IN PSUM — no DVE mask-add pass, and ScalarE's exp reads PSUM
         directly.  Computing S transposed leaves the exponentials
         already in the [j-part, i-free] layout the output matmul needs
         as stationary, eliminating the E^T PE-transposes.
  softmax: exp with fixed -60 offset (+bias folded into the exp bias
         column) on ScalarE -> Et bf16 (scores ~N(0,15.4^2): global max
         ~84 -> exp(s-60)<=e^24, row max >= ~30 -> Z >= e^-30, both in
         bf16 range; softmax is shift-invariant so no row max needed).
         The row-sum Z[i] (a partition-axis sum in this layout) comes
         from a ones-column appended to x16: the output matmul's second
         half carries one extra column accumulating exactly Z[i].
  out  = diag(1/Z) * E @ x16                  (32 mm bf16), 1/Z scale
         fused into the PSUM evacuations (DVE half1+recip, ScalarE half0).

The PE is the pacer (~20.5us/batch, structurally minimal: 705.6M MACs at
128x128/cycle + 24 transposes); other engines have ~2x headroom. x DMAs
are issued THREE batches ahead (transposes never wait on HBM), and the
x16 bf16 conversion is emitted only once its data is resident so no
engine queue convoys behind a DMA wait. DMA: 1 load op/batch on the sync
HWDGE queue, 1 store op/batch on the scalar HWDGE queue; prologue x/W
split across both queues; the last batch's stores stream per-chunk.
"""

import numpy as np

B, N, H = 64, 512, 768
NCORES = 8
BPC = B // NCORES   # batches per core
NP = 128            # SBUF partitions
NC_I = N // NP      # 4 chunks of the sequence dim
NC_H = H // NP      # 6 chunks of the hidden dim
FH = 384            # out matmul free-dim split (768 = 2*384; half1 gets +1 Z col)
NEG_BIG = -30000.0

_CACHE = {}


def _build(bpc=BPC):
    import concourse.bass as bass  # noqa: F401
    import concourse.tile as tile
    from concourse import bacc, mybir
    from concourse.bass import ts, ds

    f32 = mybir.dt.float32
    f32r = mybir.dt.float32r
    bf16 = mybir.dt.bfloat16

    nc = bacc.Bacc(
        "TRN2",
        target_bir_lowering=False,
        debug=False,
        enable_asserts=True,
        num_devices=NCORES,
    )
    x_ext = nc.dram_tensor("arg_embeddings", [bpc, N, H], f32r, kind="ExternalInput").ap()
    w_ext = nc.dram_tensor("relation_W", [H, H], f32r, kind="ExternalInput").ap()
    b_ext = nc.dram_tensor("relation_b", [1, 1], f32, kind="ExternalInput").ap()
    out_ext = nc.dram_tensor("out", [bpc, N, H], f32, kind="ExternalOutput").ap()

    HP1 = H + 1  # x16 rows carry a trailing ones-column (Z accumulator)

    with tile.TileContext(nc) as tc:
        with (
            tc.tile_pool(name="const", bufs=1) as const_pool,
            tc.tile_pool(name="w", bufs=1) as w_pool,
            tc.tile_pool(name="xnat", bufs=3) as xnat_pool,
            tc.tile_pool(name="x16", bufs=4) as x16_pool,
            tc.tile_pool(name="xT", bufs=3 * NC_H) as xT_pool,
            tc.tile_pool(name="xWt", bufs=2 * NC_H) as xWt_pool,
            tc.tile_pool(name="et", bufs=2 * NC_I) as et_pool,
            tc.tile_pool(name="stat", bufs=2 * NC_I) as stat_pool,
            tc.tile_pool(name="osb", bufs=2) as out_pool,
            tc.tile_pool(name="psT", bufs=2, space="PSUM") as psT_pool,
            tc.tile_pool(name="psA", bufs=2, space="PSUM") as psA_pool,
            tc.tile_pool(name="psS", bufs=2, space="PSUM") as psS_pool,
            tc.tile_pool(name="psC", bufs=2, space="PSUM") as psC_pool,
        ):
            # identity first — it gates batch 0's transposes
            ident_f32 = const_pool.tile([NP, NP], f32, tag="ident_f32")
            from concourse.masks import make_identity

            make_identity(nc, ident_f32[:])
            ident = const_pool.tile([NP, NP], f32r, tag="ident")
            nc.vector.tensor_copy(out=ident[:], in_=ident_f32[:])

            XNAT = {}

            def emit_dma_x(b, split_queues=False):
                x_nat = xnat_pool.tile([NP, NC_I, H], f32r, tag="xnat")
                src = x_ext[b].rearrange("(c p) h -> p c h", p=NP)
                if split_queues:
                    nc.sync.dma_start(x_nat[:, 0:2, :], src[:, 0:2, :])
                    nc.scalar.dma_start(x_nat[:, 2:4, :], src[:, 2:4, :])
                else:
                    nc.sync.dma_start(x_nat[:], src)
                XNAT[b] = x_nat

            def emit_transpose_hc(b, hc, xT):
                x_nat = XNAT[b]
                pt = psT_pool.tile([NP, N], f32r, tag="psT")
                for ic in range(NC_I):
                    nc.tensor.matmul(
                        pt[:, ts(ic, NP)],
                        x_nat[:, ic, ts(hc, NP)],
                        ident[:],
                        is_transpose=True,
                        start=(ic == 0),
                        stop=(ic == NC_I - 1),
                    )
                xt = xT_pool.tile([NP, N], f32r, tag="xT")
                nc.scalar.copy(out=xt[:], in_=pt[:])
                xT.append(xt)

            def emit_x16(b):
                x_nat = XNAT.pop(b)
                x16 = x16_pool.tile([NP, NC_I, HP1], bf16, tag="x16")
                nc.vector.tensor_copy(out=x16[:, :, 0:H], in_=x_nat[:])
                nc.gpsimd.memset(x16[:, :, H : H + 1], 1.0)
                return x16

            def emit_consts():
                # one-hot slabs for the PE diagonal mask: islab[jc][m, i] = 1
                # where i == jc*128 + m; negident = NEG_BIG * I.  mmB's 25th
                # accumulation matmul negident.T @ islab[jc] lands NEG_BIG on
                # the diagonal of S^T in PSUM.  (f32r tensors must be produced
                # by copy/DMA — memset/affine_select emit f32, then convert.)
                islabs_f = const_pool.tile([NP, NC_I, N], f32, tag="islabs_f")
                nc.vector.memset(islabs_f[:], 0.0)
                for jc in range(NC_I):
                    nc.gpsimd.affine_select(
                        out=islabs_f[:, jc, :],
                        in_=islabs_f[:, jc, :],
                        compare_op=mybir.AluOpType.not_equal,
                        fill=1.0,
                        base=jc * NP,
                        channel_multiplier=1,
                        pattern=[[-1, N]],
                    )
                islabs = const_pool.tile([NP, NC_I, N], f32r, tag="islabs")
                nc.vector.tensor_copy(out=islabs[:], in_=islabs_f[:])
                negident_f = const_pool.tile([NP, NP], f32, tag="negident_f")
                nc.vector.memset(negident_f[:], 0.0)
                nc.gpsimd.affine_select(
                    out=negident_f[:],
                    in_=negident_f[:],
                    compare_op=mybir.AluOpType.not_equal,
                    fill=NEG_BIG,
                    base=0,
                    channel_multiplier=1,
                    pattern=[[-1, NP]],
                )
                negident = const_pool.tile([NP, NP], f32r, tag="negident")
                nc.vector.tensor_copy(out=negident[:], in_=negident_f[:])
                # exp bias column: bias - 60 (fixed softmax stability shift)
                b_row = const_pool.tile([1, 1], f32, tag="brow")
                nc.sync.dma_start(b_row[:], b_ext[:])
                b_col = const_pool.tile([NP, 1], f32, tag="bcol")
                nc.gpsimd.partition_broadcast(b_col[:], b_row[:])
                neg60b = const_pool.tile([NP, 1], f32, tag="neg60b")
                nc.vector.memset(neg60b[:], -60.0)
                nc.vector.tensor_add(neg60b[:], neg60b[:], b_col[:])
                C["neg60b"] = neg60b
                C["islabs"] = islabs
                C["negident"] = negident

            C = {}

            def emit_w():
                w16 = w_pool.tile([NP, NC_H, H], f32r, tag="w16")
                C["w16"] = w16
                for hc in range(NC_H):
                    eng = nc.sync if hc % 2 == 0 else nc.scalar
                    eng.dma_start(w16[:, hc, :], w_ext[ts(hc, NP), :])

            def emit_mmA_kc(b, xT, kc, xWt):
                w16 = C["w16"]
                # xWt[kc][p, i] = sum_h W[h, kc*128+p] * x[i, h]
                ps = psA_pool.tile([NP, N], f32, tag="psA")
                for hc in range(NC_H):
                    nc.tensor.matmul(
                        ps[:],
                        w16[:, hc, ts(kc, NP)],
                        xT[hc][:],
                        start=(hc == 0),
                        stop=(hc == NC_H - 1),
                    )
                xw = xWt_pool.tile([NP, N], f32r, tag="xWt")
                nc.vector.tensor_copy(out=xw[:], in_=ps[:])
                xWt.append(xw)

            def emit_mmB_jc(b, xT, xWt, jc, ET):
                # S^T chunk jc: St[p, i] = sum_k xT[k, jc*128+p] * xWt[k, i]
                # + NEG_BIG on the diagonal (25th matmul, in PSUM)
                ps = psS_pool.tile([NP, N], f32, tag="psS")
                for kc in range(NC_H):
                    nc.tensor.matmul(
                        ps[:],
                        xT[kc][:, ts(jc, NP)],
                        xWt[kc][:],
                        start=(kc == 0),
                        stop=False,
                    )
                nc.tensor.matmul(
                    ps[:],
                    C["negident"][:],
                    C["islabs"][:, jc, :],
                    start=False,
                    stop=True,
                )
                # exp reads S^T straight from PSUM
                et = et_pool.tile([NP, N], bf16, tag="et")
                nc.scalar.activation(
                    et[:],
                    ps[:],
                    mybir.ActivationFunctionType.Exp,
                    bias=C["neg60b"][:],
                    scale=1.0,
                )
                ET.append(et)

            def emit_finalize_ic(st, ic, last=False):
                b, x16, ET, osb = st["b"], st["x16"], st["ET"], st["osb"]
                # out chunk ic: out[p, h] = (1/Z[p]) * sum_j E[ic*128+p, j] x[j, h]
                # half1 carries the ones-column whose accumulated value is
                # Z[p]; its reciprocal scales both halves' evacuation.
                ps1 = psC_pool.tile([NP, FH + 1], f32, tag="psC")
                for jc in range(NC_I):
                    nc.tensor.matmul(
                        ps1[:],
                        ET[jc][:, ts(ic, NP)],
                        x16[:, jc, ds(FH, FH + 1)],
                        start=(jc == 0),
                        stop=(jc == NC_I - 1),
                    )
                r = stat_pool.tile([NP, 1], f32, tag="r")
                nc.vector.reciprocal(r[:], ps1[:, FH : FH + 1])
                nc.vector.tensor_scalar_mul(osb[:, ic, ds(FH, FH)], ps1[:, 0:FH], r[:])
                ps0 = psC_pool.tile([NP, FH + 1], f32, tag="psC")
                for jc in range(NC_I):
                    nc.tensor.matmul(
                        ps0[:, 0:FH],
                        ET[jc][:, ts(ic, NP)],
                        x16[:, jc, ds(0, FH)],
                        start=(jc == 0),
                        stop=(jc == NC_I - 1),
                    )
                nc.scalar.activation(
                    osb[:, ic, ds(0, FH)],
                    ps0[:, 0:FH],
                    mybir.ActivationFunctionType.Copy,
                    scale=r[:],
                )
                if last:
                    # stream the last batch's output per-chunk so the final
                    # store overlaps the remaining finalize work
                    nc.scalar.dma_start(out_ext[b][ts(ic, NP), :], osb[:, ic, :])
                elif ic == NC_I - 1:
                    nc.scalar.dma_start(
                        out_ext[b].rearrange("(c p) h -> p c h", p=NP), osb[:]
                    )

            # Prologue: x(0..2) + W split across both HWDGE queues, consts,
            # transposes(0) and (1) standalone, x16(0) and (1).
            emit_dma_x(0, split_queues=True)
            emit_w()
            emit_dma_x(1, split_queues=True)
            emit_consts()
            if bpc > 2:
                emit_dma_x(2, split_queues=True)
            xTs = {0: [], 1: []}
            for hc in range(NC_H):
                emit_transpose_hc(0, hc, xTs[0])
            X16 = {0: emit_x16(0)}
            for hc in range(NC_H):
                emit_transpose_hc(1, hc, xTs[1])
            X16[1] = emit_x16(1)

            # Steady state, iteration b:
            #   dma_x(b+3) | mmA(b) interleaved with transposes(b+2) |
            #   x16(b+2) | finalize(b-1) | mmB(b)
            prev = None
            for b in range(bpc):
                if b + 3 < bpc:
                    emit_dma_x(b + 3)
                xT = xTs.pop(b)
                xWt = []
                tgt = xTs.setdefault(b + 2, []) if b + 2 < bpc else None
                for kc in range(NC_H):
                    emit_mmA_kc(b, xT, kc, xWt)
                    if tgt is not None:
                        emit_transpose_hc(b + 2, kc, tgt)
                if b + 2 < bpc:
                    X16[b + 2] = emit_x16(b + 2)
                x16 = X16.pop(b)
                osb = out_pool.tile([NP, NC_I, H], f32, tag="osb")
                if prev is not None:
                    for ic in range(NC_I):
                        emit_finalize_ic(prev, ic)
                ET = []
                for jc in range(NC_I):
                    emit_mmB_jc(b, xT, xWt, jc, ET)
                prev = {"b": b, "x16": x16, "ET": ET, "osb": osb}
            for ic in range(NC_I):
                emit_finalize_ic(prev, ic, last=True)

    nc.compile()
    return nc


def _get_nc(bpc=BPC):
    if bpc not in _CACHE:
        _CACHE[bpc] = _build(bpc)
    return _CACHE[bpc]


def make_in_maps(arg_embeddings, relation_W, relation_b, bpc=BPC):
    x = np.ascontiguousarray(arg_embeddings, dtype=np.float32)
    W = np.ascontiguousarray(relation_W, dtype=np.float32)
    bb = np.asarray(relation_b, dtype=np.float32).reshape(1, 1)
    return [
        {
            "arg_embeddings": np.ascontiguousarray(x[c * bpc : (c + 1) * bpc]),
            "relation_W": W,
            "relation_b": bb,
        }
        for c in range(NCORES)
    ]


def kernel(arg_embeddings, relation_W, relation_b):
    from concourse.bass_utils import run_bass_kernel_spmd

    nc = _get_nc()
    in_maps = make_in_maps(arg_embeddings, relation_W, relation_b)
    res = run_bass_kernel_spmd(nc, in_maps, core_ids=list(range(NCORES)))
    out = np.concatenate([res.results[c]["out"] for c in range(NCORES)], axis=0)
    return np.ascontiguousarray(out, dtype=np.float32)


# revision 13
# speedup vs baseline: 1.0083x; 1.0083x over previous
"""Trainium2 Bass kernel: ArgumentRelationAttention.

out[b] = softmax_j(mask_diag(x[b] @ W @ x[b]^T + bias)) @ x[b]
  x: [64, 512, 768] f32, W: [768, 768] f32, bias: [1] f32

Strategy: pure batch data parallelism — 8 batches per NeuronCore x 8 cores.
Per batch everything stays on-chip. The PE streams ~1 col/cycle for every
dtype (measured: 227ns f32r / 216ns bf16 per 512-free matmul), so the
score path stays f32r (full precision, zero dtype conversions — DMA
writes f32r directly) and only the softmax/output stage drops to bf16:

  xT   = PE-transpose(x), f32r, 4 transposes per PSUM bank
  xWt[k,i] = sum_h W[h,k] xT[h,i]            (36 mm f32r)
  S^T  = scores TRANSPOSED: stationary xT[:, jchunk], moving xWt
         -> St[j, i] (24 mm); a 25th accumulation matmul per chunk
         (lhsT = -30000*I, rhs = one-hot slab) adds the diagonal mask
         IN PSUM — no DVE mask-add pass, and ScalarE's exp reads PSUM
         directly.  Computing S transposed leaves the exponentials
         already in the [j-part, i-free] layout the output matmul needs
         as stationary, eliminating the E^T PE-transposes.
  softmax: exp with fixed -60 offset (+bias folded into the exp bias
         column) on ScalarE -> Et bf16 (scores ~N(0,15.4^2): global max
         ~84 -> exp(s-60)<=e^24, row max >= ~30 -> Z >= e^-30, both in
         bf16 range; softmax is shift-invariant so no row max needed).
         The row-sum Z[i] (a partition-axis sum in this layout) comes
         from a ones-column appended to x16: the output matmul's second
         half carries one extra column accumulating exactly Z[i].
  out  = diag(1/Z) * E @ x16                  (32 mm bf16), 1/Z scale
         fused into the PSUM evacuations (DVE/ScalarE alternating).

The PE is the pacer (~20.5us/batch, structurally minimal: 705.6M MACs at
128x128/cycle + 24 transposes); all other engines have ~2x headroom so
the software pipeline (loads+transposes 2 batches ahead, finalize(b-1)
between mmA(b) and mmB(b)) keeps PE occupancy ~100% mid-stream. DMA is
consolidated to 1 load + 1 store op per batch (x loads on the sync HWDGE
queue, stores + W on the scalar HWDGE queue); batch 0/1 loads and W
split across both queues to shorten the ramp, and the last batch's
stores stream per-chunk to shorten the tail.
"""

import numpy as np

B, N, H = 64, 512, 768
NCORES = 8
BPC = B // NCORES   # batches per core
NP = 128            # SBUF partitions
NC_I = N // NP      # 4 chunks of the sequence dim
NC_H = H // NP      # 6 chunks of the hidden dim
FH = 384            # out matmul free-dim split (768 = 2*384; half1 gets +1 Z col)
NEG_BIG = -30000.0

_CACHE = {}


def _build(bpc=BPC):
    import concourse.bass as bass  # noqa: F401
    import concourse.tile as tile
    from concourse import bacc, mybir
    from concourse.bass import ts, ds

    f32 = mybir.dt.float32
    f32r = mybir.dt.float32r
    bf16 = mybir.dt.bfloat16

    nc = bacc.Bacc(
        "TRN2",
        target_bir_lowering=False,
        debug=False,
        enable_asserts=True,
        num_devices=NCORES,
    )
    x_ext = nc.dram_tensor("arg_embeddings", [bpc, N, H], f32r, kind="ExternalInput").ap()
    w_ext = nc.dram_tensor("relation_W", [H, H], f32r, kind="ExternalInput").ap()
    b_ext = nc.dram_tensor("relation_b", [1, 1], f32, kind="ExternalInput").ap()
    out_ext = nc.dram_tensor("out", [bpc, N, H], f32, kind="ExternalOutput").ap()

    HP1 = H + 1  # x16 rows carry a trailing ones-column (Z accumulator)

    with tile.TileContext(nc) as tc:
        with (
            tc.tile_pool(name="const", bufs=1) as const_pool,
            tc.tile_pool(name="w", bufs=1) as w_pool,
            tc.tile_pool(name="xnat", bufs=3) as xnat_pool,
            tc.tile_pool(name="x16", bufs=4) as x16_pool,
            tc.tile_pool(name="xT", bufs=3 * NC_H) as xT_pool,
            tc.tile_pool(name="xWt", bufs=2 * NC_H) as xWt_pool,
            tc.tile_pool(name="et", bufs=2 * NC_I) as et_pool,
            tc.tile_pool(name="stat", bufs=2 * NC_I) as stat_pool,
            tc.tile_pool(name="osb", bufs=2) as out_pool,
            tc.tile_pool(name="psT", bufs=2, space="PSUM") as psT_pool,
            tc.tile_pool(name="psA", bufs=2, space="PSUM") as psA_pool,
            tc.tile_pool(name="psS", bufs=2, space="PSUM") as psS_pool,
            tc.tile_pool(name="psC", bufs=2, space="PSUM") as psC_pool,
        ):
            # identity first — it gates batch 0's transposes
            ident_f32 = const_pool.tile([NP, NP], f32, tag="ident_f32")
            from concourse.masks import make_identity

            make_identity(nc, ident_f32[:])
            ident = const_pool.tile([NP, NP], f32r, tag="ident")
            nc.vector.tensor_copy(out=ident[:], in_=ident_f32[:])

            def emit_load(b, split_queues=False):
                x_nat = xnat_pool.tile([NP, NC_I, H], f32r, tag="xnat")
                src = x_ext[b].rearrange("(c p) h -> p c h", p=NP)
                if split_queues:
                    nc.sync.dma_start(x_nat[:, 0:2, :], src[:, 0:2, :])
                    nc.scalar.dma_start(x_nat[:, 2:4, :], src[:, 2:4, :])
                else:
                    nc.sync.dma_start(x_nat[:], src)
                # x^T chunks via PE transposes, 4 per PSUM bank
                xT = []
                for hc in range(NC_H):
                    pt = psT_pool.tile([NP, N], f32r, tag="psT")
                    for ic in range(NC_I):
                        nc.tensor.matmul(
                            pt[:, ts(ic, NP)],
                            x_nat[:, ic, ts(hc, NP)],
                            ident[:],
                            is_transpose=True,
                            start=(ic == 0),
                            stop=(ic == NC_I - 1),
                        )
                    xt = xT_pool.tile([NP, N], f32r, tag="xT")
                    nc.scalar.copy(out=xt[:], in_=pt[:])
                    xT.append(xt)
                return x_nat, xT

            def emit_x16(x_nat):
                # bf16 copy of x (+ones column) for the output matmul.
                # Emitted AFTER finalize(b-1)'s DVE ops: its consumer is a
                # full iteration away, and placing it earlier would convoy
                # the finalize reciprocal/scale ops behind it in the DVE
                # queue (measured ~1us/batch of psC stalls).
                x16 = x16_pool.tile([NP, NC_I, HP1], bf16, tag="x16")
                nc.vector.tensor_copy(out=x16[:, :, 0:H], in_=x_nat[:])
                nc.gpsimd.memset(x16[:, :, H : H + 1], 1.0)
                return x16

            def emit_consts():
                # one-hot slabs for the PE diagonal mask: islab[jc][m, i] = 1
                # where i == jc*128 + m; negident = NEG_BIG * I.  mmB's 25th
                # accumulation matmul negident.T @ islab[jc] lands NEG_BIG on
                # the diagonal of S^T in PSUM.
                islabs_f = const_pool.tile([NP, NC_I, N], f32, tag="islabs_f")
                nc.vector.memset(islabs_f[:], 0.0)
                for jc in range(NC_I):
                    nc.gpsimd.affine_select(
                        out=islabs_f[:, jc, :],
                        in_=islabs_f[:, jc, :],
                        compare_op=mybir.AluOpType.not_equal,
                        fill=1.0,
                        base=jc * NP,
                        channel_multiplier=1,
                        pattern=[[-1, N]],
                    )
                islabs = const_pool.tile([NP, NC_I, N], f32r, tag="islabs")
                nc.vector.tensor_copy(out=islabs[:], in_=islabs_f[:])
                negident_f = const_pool.tile([NP, NP], f32, tag="negident_f")
                nc.vector.memset(negident_f[:], 0.0)
                nc.gpsimd.affine_select(
                    out=negident_f[:],
                    in_=negident_f[:],
                    compare_op=mybir.AluOpType.not_equal,
                    fill=NEG_BIG,
                    base=0,
                    channel_multiplier=1,
                    pattern=[[-1, NP]],
                )
                negident = const_pool.tile([NP, NP], f32r, tag="negident")
                nc.vector.tensor_copy(out=negident[:], in_=negident_f[:])
                # exp bias column: bias - 60 (fixed softmax stability shift)
                b_row = const_pool.tile([1, 1], f32, tag="brow")
                nc.sync.dma_start(b_row[:], b_ext[:])
                b_col = const_pool.tile([NP, 1], f32, tag="bcol")
                nc.gpsimd.partition_broadcast(b_col[:], b_row[:])
                neg60b = const_pool.tile([NP, 1], f32, tag="neg60b")
                nc.vector.memset(neg60b[:], -60.0)
                nc.vector.tensor_add(neg60b[:], neg60b[:], b_col[:])
                C["neg60b"] = neg60b
                C["islabs"] = islabs
                C["negident"] = negident

            C = {}

            def emit_w():
                w16 = w_pool.tile([NP, NC_H, H], f32r, tag="w16")
                C["w16"] = w16
                for hc in range(NC_H):
                    eng = nc.sync if hc % 2 == 0 else nc.scalar
                    eng.dma_start(w16[:, hc, :], w_ext[ts(hc, NP), :])

            def emit_mmA(b, xT):
                w16 = C["w16"]
                # xWt[kc][p, i] = sum_h W[h, kc*128+p] * x[i, h]
                xWt = []
                for kc in range(NC_H):
                    ps = psA_pool.tile([NP, N], f32, tag="psA")
                    for hc in range(NC_H):
                        nc.tensor.matmul(
                            ps[:],
                            w16[:, hc, ts(kc, NP)],
                            xT[hc][:],
                            start=(hc == 0),
                            stop=(hc == NC_H - 1),
                        )
                    xw = xWt_pool.tile([NP, N], f32r, tag="xWt")
                    nc.vector.tensor_copy(out=xw[:], in_=ps[:])
                    xWt.append(xw)
                return xWt

            def emit_mmB_jc(b, xT, xWt, jc, ET):
                # S^T chunk jc: St[p, i] = sum_k xT[k, jc*128+p] * xWt[k, i]
                # + NEG_BIG on the diagonal (25th matmul, in PSUM)
                ps = psS_pool.tile([NP, N], f32, tag="psS")
                for kc in range(NC_H):
                    nc.tensor.matmul(
                        ps[:],
                        xT[kc][:, ts(jc, NP)],
                        xWt[kc][:],
                        start=(kc == 0),
                        stop=False,
                    )
                nc.tensor.matmul(
                    ps[:],
                    C["negident"][:],
                    C["islabs"][:, jc, :],
                    start=False,
                    stop=True,
                )
                # exp reads S^T straight from PSUM
                et = et_pool.tile([NP, N], bf16, tag="et")
                nc.scalar.activation(
                    et[:],
                    ps[:],
                    mybir.ActivationFunctionType.Exp,
                    bias=C["neg60b"][:],
                    scale=1.0,
                )
                ET.append(et)

            def emit_finalize_ic(st, ic, last=False):
                b, x16, ET, osb = st["b"], st["x16"], st["ET"], st["osb"]
                # out chunk ic: out[p, h] = (1/Z[p]) * sum_j E[ic*128+p, j] x[j, h]
                # half1 carries the ones-column whose accumulated value is
                # Z[p]; its reciprocal scales both halves' evacuation.
                ps1 = psC_pool.tile([NP, FH + 1], f32, tag="psC")
                for jc in range(NC_I):
                    nc.tensor.matmul(
                        ps1[:],
                        ET[jc][:, ts(ic, NP)],
                        x16[:, jc, ds(FH, FH + 1)],
                        start=(jc == 0),
                        stop=(jc == NC_I - 1),
                    )
                r = stat_pool.tile([NP, 1], f32, tag="r")
                nc.vector.reciprocal(r[:], ps1[:, FH : FH + 1])
                nc.vector.tensor_scalar_mul(osb[:, ic, ds(FH, FH)], ps1[:, 0:FH], r[:])
                ps0 = psC_pool.tile([NP, FH + 1], f32, tag="psC")
                for jc in range(NC_I):
                    nc.tensor.matmul(
                        ps0[:, 0:FH],
                        ET[jc][:, ts(ic, NP)],
                        x16[:, jc, ds(0, FH)],
                        start=(jc == 0),
                        stop=(jc == NC_I - 1),
                    )
                nc.scalar.activation(
                    osb[:, ic, ds(0, FH)],
                    ps0[:, 0:FH],
                    mybir.ActivationFunctionType.Copy,
                    scale=r[:],
                )
                if last:
                    # stream the last batch's output per-chunk so the final
                    # store overlaps the remaining finalize work
                    nc.scalar.dma_start(out_ext[b][ts(ic, NP), :], osb[:, ic, :])
                elif ic == NC_I - 1:
                    nc.scalar.dma_start(
                        out_ext[b].rearrange("(c p) h -> p c h", p=NP), osb[:]
                    )

            # Emission order = scheduler priority. Batch 0/1 x loads and the
            # W chunks split across both HWDGE queues to shorten the ramp.
            # Steady-state PE order per iteration: mmA(b), transposes(b+2),
            # finalize(b-1), mmB(b) — transposes + finalize hide the xWt
            # evacuation latency so mmB never stalls.
            loads = {0: emit_load(0, split_queues=True)}
            emit_w()
            if bpc > 1:
                loads[1] = emit_load(1, split_queues=True)
            emit_consts()
            prev = None
            for b in range(bpc):
                x_nat, xT = loads.pop(b)
                xWt = emit_mmA(b, xT)
                if b + 2 < bpc:
                    loads[b + 2] = emit_load(b + 2)
                osb = out_pool.tile([NP, NC_I, H], f32, tag="osb")
                if prev is not None:
                    for ic in range(NC_I):
                        emit_finalize_ic(prev, ic)
                x16 = emit_x16(x_nat)
                ET = []
                for jc in range(NC_I):
                    emit_mmB_jc(b, xT, xWt, jc, ET)
                prev = {"b": b, "x16": x16, "ET": ET, "osb": osb}
            for ic in range(NC_I):
                emit_finalize_ic(prev, ic, last=True)

    nc.compile()
    return nc


def _get_nc(bpc=BPC):
    if bpc not in _CACHE:
        _CACHE[bpc] = _build(bpc)
    return _CACHE[bpc]


def make_in_maps(arg_embeddings, relation_W, relation_b, bpc=BPC):
    x = np.ascontiguousarray(arg_embeddings, dtype=np.float32)
    W = np.ascontiguousarray(relation_W, dtype=np.float32)
    bb = np.asarray(relation_b, dtype=np.float32).reshape(1, 1)
    return [
        {
            "arg_embeddings": np.ascontiguousarray(x[c * bpc : (c + 1) * bpc]),
            "relation_W": W,
            "relation_b": bb,
        }
        for c in range(NCORES)
    ]


def kernel(arg_embeddings, relation_W, relation_b):
    from concourse.bass_utils import run_bass_kernel_spmd

    nc = _get_nc()
    in_maps = make_in_maps(arg_embeddings, relation_W, relation_b)
    res = run_bass_kernel_spmd(nc, in_maps, core_ids=list(range(NCORES)))
    out = np.concatenate([res.results[c]["out"] for c in range(NCORES)], axis=0)
    return np.ascontiguousarray(out, dtype=np.float32)


# revision 15
# speedup vs baseline: 1.0137x; 1.0054x over previous
"""Trainium2 Bass kernel: ArgumentRelationAttention.

out[b] = softmax_j(mask_diag(x[b] @ W @ x[b]^T + bias)) @ x[b]
  x: [64, 512, 768] f32, W: [768, 768] f32, bias: [1] f32

Strategy: pure batch data parallelism — 8 batches per NeuronCore x 8 cores.
Per batch everything stays on-chip, all matmuls in bf16 (measured 216ns
vs 227ns f32r per 512-free matmul, and bf16 transposes get fast weight
load): a single f32->bf16 conversion of x feeds transposes, scores and
the output matmul (numerically validated 1.29e-2 rel err, gate 2e-2):

  x16  = bf16(x) + trailing ones-column
  xT   = PE-transpose(x16), 4 transposes per PSUM bank
  xWt[k,i] = sum_h W16[h,k] xT[h,i]          (36 mm)
  S^T  = scores TRANSPOSED: stationary xT[:, jchunk], moving xWt
         -> St[j, i] (24 mm); a 25th accumulation matmul per chunk
         (lhsT = -30000*I, rhs = one-hot slab) adds the diagonal mask
         IN PSUM — no DVE mask-add pass, and ScalarE's exp reads PSUM
         directly.  Computing S transposed leaves the exponentials
         already in the [j-part, i-free] layout the output matmul needs
         as stationary, eliminating the E^T PE-transposes.
  softmax: exp with fixed -60 offset (+bias folded into the exp bias
         column) on ScalarE -> Et bf16 (scores ~N(0,15.4^2): global max
         ~84 -> exp(s-60)<=e^24, row max >= ~30 -> Z >= e^-30, both in
         bf16 range; softmax is shift-invariant so no row max needed).
         The row-sum Z[i] (a partition-axis sum in this layout) comes
         from a ones-column appended to x16: the output matmul's second
         half carries one extra column accumulating exactly Z[i].
  out  = diag(1/Z) * E @ x16                  (32 mm bf16), 1/Z scale
         fused into the PSUM evacuations (DVE/ScalarE alternating).

The PE is the pacer (~20.5us/batch, structurally minimal: 705.6M MACs at
128x128/cycle + 24 transposes); all other engines have ~2x headroom so
the software pipeline (loads+transposes 2 batches ahead, finalize(b-1)
between mmA(b) and mmB(b)) keeps PE occupancy ~100% mid-stream. DMA is
consolidated to 1 load + 1 store op per batch (x loads on the sync HWDGE
queue, stores + W on the scalar HWDGE queue); batch 0/1 loads and W
split across both queues to shorten the ramp, and the last batch's
stores stream per-chunk to shorten the tail.
"""

import numpy as np

B, N, H = 64, 512, 768
NCORES = 8
BPC = B // NCORES   # batches per core
NP = 128            # SBUF partitions
NC_I = N // NP      # 4 chunks of the sequence dim
NC_H = H // NP      # 6 chunks of the hidden dim
FH = 384            # out matmul free-dim split (768 = 2*384; half1 gets +1 Z col)
NEG_BIG = -30000.0

_CACHE = {}


def _build(bpc=BPC):
    import concourse.bass as bass  # noqa: F401
    import concourse.tile as tile
    from concourse import bacc, mybir
    from concourse.bass import ts, ds

    f32 = mybir.dt.float32
    f32r = mybir.dt.float32r
    bf16 = mybir.dt.bfloat16

    nc = bacc.Bacc(
        "TRN2",
        target_bir_lowering=False,
        debug=False,
        enable_asserts=True,
        num_devices=NCORES,
    )
    x_ext = nc.dram_tensor("arg_embeddings", [bpc, N, H], f32, kind="ExternalInput").ap()
    w_ext = nc.dram_tensor("relation_W", [H, H], f32, kind="ExternalInput").ap()
    b_ext = nc.dram_tensor("relation_b", [1, 1], f32, kind="ExternalInput").ap()
    out_ext = nc.dram_tensor("out", [bpc, N, H], f32, kind="ExternalOutput").ap()

    HP1 = H + 1  # x16 rows carry a trailing ones-column (Z accumulator)

    with tile.TileContext(nc) as tc:
        with (
            tc.tile_pool(name="const", bufs=1) as const_pool,
            tc.tile_pool(name="w", bufs=1) as w_pool,
            tc.tile_pool(name="wstage", bufs=2) as wstage_pool,
            tc.tile_pool(name="xnat", bufs=3) as xnat_pool,
            tc.tile_pool(name="x16", bufs=4) as x16_pool,
            tc.tile_pool(name="xT", bufs=3 * NC_H) as xT_pool,
            tc.tile_pool(name="xWt", bufs=2 * NC_H) as xWt_pool,
            tc.tile_pool(name="et", bufs=2 * NC_I) as et_pool,
            tc.tile_pool(name="stat", bufs=2 * NC_I) as stat_pool,
            tc.tile_pool(name="osb", bufs=2) as out_pool,
            tc.tile_pool(name="psT", bufs=2, space="PSUM") as psT_pool,
            tc.tile_pool(name="psA", bufs=2, space="PSUM") as psA_pool,
            tc.tile_pool(name="psS", bufs=2, space="PSUM") as psS_pool,
            tc.tile_pool(name="psC", bufs=2, space="PSUM") as psC_pool,
        ):
            # identity first — it gates batch 0's transposes
            ident_f32 = const_pool.tile([NP, NP], f32, tag="ident_f32")
            from concourse.masks import make_identity

            make_identity(nc, ident_f32[:])
            ident = const_pool.tile([NP, NP], bf16, tag="ident")
            nc.vector.tensor_copy(out=ident[:], in_=ident_f32[:])

            def emit_load(b, split_queues=False):
                x_nat = xnat_pool.tile([NP, NC_I, H], f32, tag="xnat")
                src = x_ext[b].rearrange("(c p) h -> p c h", p=NP)
                if split_queues:
                    nc.sync.dma_start(x_nat[:, 0:2, :], src[:, 0:2, :])
                    nc.scalar.dma_start(x_nat[:, 2:4, :], src[:, 2:4, :])
                else:
                    nc.sync.dma_start(x_nat[:], src)
                # single bf16 conversion feeds transposes, scores and the
                # output matmul (ones column appended for the Z accumulator)
                x16 = x16_pool.tile([NP, NC_I, HP1], bf16, tag="x16")
                nc.vector.tensor_copy(out=x16[:, :, 0:H], in_=x_nat[:])
                nc.gpsimd.memset(x16[:, :, H : H + 1], 1.0)
                # x^T chunks via PE transposes (bf16), 4 per PSUM bank
                xT = []
                for hc in range(NC_H):
                    pt = psT_pool.tile([NP, N], bf16, tag="psT")
                    for ic in range(NC_I):
                        nc.tensor.matmul(
                            pt[:, ts(ic, NP)],
                            x16[:, ic, ts(hc, NP)],
                            ident[:],
                            is_transpose=True,
                            start=(ic == 0),
                            stop=(ic == NC_I - 1),
                        )
                    xt = xT_pool.tile([NP, N], bf16, tag="xT")
                    nc.scalar.copy(out=xt[:], in_=pt[:])
                    xT.append(xt)
                return x16, xT

            def emit_consts():
                # one-hot slabs for the PE diagonal mask: islab[jc][m, i] = 1
                # where i == jc*128 + m; negident = NEG_BIG * I.  mmB's 25th
                # accumulation matmul negident.T @ islab[jc] lands NEG_BIG on
                # the diagonal of S^T in PSUM.
                islabs = const_pool.tile([NP, NC_I, N], bf16, tag="islabs")
                nc.vector.memset(islabs[:], 0.0)
                for jc in range(NC_I):
                    nc.gpsimd.affine_select(
                        out=islabs[:, jc, :],
                        in_=islabs[:, jc, :],
                        compare_op=mybir.AluOpType.not_equal,
                        fill=1.0,
                        base=jc * NP,
                        channel_multiplier=1,
                        pattern=[[-1, N]],
                    )
                negident = const_pool.tile([NP, NP], bf16, tag="negident")
                nc.vector.memset(negident[:], 0.0)
                nc.gpsimd.affine_select(
                    out=negident[:],
                    in_=negident[:],
                    compare_op=mybir.AluOpType.not_equal,
                    fill=NEG_BIG,
                    base=0,
                    channel_multiplier=1,
                    pattern=[[-1, NP]],
                )
                # exp bias column: bias - 60 (fixed softmax stability shift)
                b_row = const_pool.tile([1, 1], f32, tag="brow")
                nc.sync.dma_start(b_row[:], b_ext[:])
                b_col = const_pool.tile([NP, 1], f32, tag="bcol")
                nc.gpsimd.partition_broadcast(b_col[:], b_row[:])
                neg60b = const_pool.tile([NP, 1], f32, tag="neg60b")
                nc.vector.memset(neg60b[:], -60.0)
                nc.vector.tensor_add(neg60b[:], neg60b[:], b_col[:])
                C["neg60b"] = neg60b
                C["islabs"] = islabs
                C["negident"] = negident

            C = {}

            def emit_w():
                w16 = w_pool.tile([NP, NC_H, H], bf16, tag="w16")
                C["w16"] = w16
                for hc in range(NC_H):
                    stage = wstage_pool.tile([NP, H], f32, tag="wstage")
                    eng = nc.sync if hc % 2 == 0 else nc.scalar
                    eng.dma_start(stage[:], w_ext[ts(hc, NP), :])
                    nc.vector.tensor_copy(out=w16[:, hc, :], in_=stage[:])

            def emit_mmA(b, xT):
                w16 = C["w16"]
                # xWt[kc][p, i] = sum_h W[h, kc*128+p] * x[i, h]
                xWt = []
                for kc in range(NC_H):
                    ps = psA_pool.tile([NP, N], f32, tag="psA")
                    for hc in range(NC_H):
                        nc.tensor.matmul(
                            ps[:],
                            w16[:, hc, ts(kc, NP)],
                            xT[hc][:],
                            start=(hc == 0),
                            stop=(hc == NC_H - 1),
                        )
                    xw = xWt_pool.tile([NP, N], bf16, tag="xWt")
                    nc.vector.tensor_copy(out=xw[:], in_=ps[:])
                    xWt.append(xw)
                return xWt

            def emit_mmB_jc(b, xT, xWt, jc, ET):
                # S^T chunk jc: St[p, i] = sum_k xT[k, jc*128+p] * xWt[k, i]
                # + NEG_BIG on the diagonal (25th matmul, in PSUM)
                ps = psS_pool.tile([NP, N], f32, tag="psS")
                for kc in range(NC_H):
                    nc.tensor.matmul(
                        ps[:],
                        xT[kc][:, ts(jc, NP)],
                        xWt[kc][:],
                        start=(kc == 0),
                        stop=False,
                    )
                nc.tensor.matmul(
                    ps[:],
                    C["negident"][:],
                    C["islabs"][:, jc, :],
                    start=False,
                    stop=True,
                )
                # exp reads S^T straight from PSUM
                et = et_pool.tile([NP, N], bf16, tag="et")
                nc.scalar.activation(
                    et[:],
                    ps[:],
                    mybir.ActivationFunctionType.Exp,
                    bias=C["neg60b"][:],
                    scale=1.0,
                )
                ET.append(et)

            def emit_finalize_ic(st, ic, last=False):
                b, x16, ET, osb = st["b"], st["x16"], st["ET"], st["osb"]
                # out chunk ic: out[p, h] = (1/Z[p]) * sum_j E[ic*128+p, j] x[j, h]
                # half1 carries the ones-column whose accumulated value is
                # Z[p]; its reciprocal scales both halves' evacuation.
                ps1 = psC_pool.tile([NP, FH + 1], f32, tag="psC")
                for jc in range(NC_I):
                    nc.tensor.matmul(
                        ps1[:],
                        ET[jc][:, ts(ic, NP)],
                        x16[:, jc, ds(FH, FH + 1)],
                        start=(jc == 0),
                        stop=(jc == NC_I - 1),
                    )
                r = stat_pool.tile([NP, 1], f32, tag="r")
                nc.vector.reciprocal(r[:], ps1[:, FH : FH + 1])
                nc.vector.tensor_scalar_mul(osb[:, ic, ds(FH, FH)], ps1[:, 0:FH], r[:])
                ps0 = psC_pool.tile([NP, FH + 1], f32, tag="psC")
                for jc in range(NC_I):
                    nc.tensor.matmul(
                        ps0[:, 0:FH],
                        ET[jc][:, ts(ic, NP)],
                        x16[:, jc, ds(0, FH)],
                        start=(jc == 0),
                        stop=(jc == NC_I - 1),
                    )
                nc.scalar.activation(
                    osb[:, ic, ds(0, FH)],
                    ps0[:, 0:FH],
                    mybir.ActivationFunctionType.Copy,
                    scale=r[:],
                )
                if last:
                    # stream the last batch's output per-chunk so the final
                    # store overlaps the remaining finalize work
                    nc.scalar.dma_start(out_ext[b][ts(ic, NP), :], osb[:, ic, :])
                elif ic == NC_I - 1:
                    nc.scalar.dma_start(
                        out_ext[b].rearrange("(c p) h -> p c h", p=NP), osb[:]
                    )

            # Emission order = scheduler priority. Batch 0/1 x loads and the
            # W chunks split across both HWDGE queues to shorten the ramp.
            # Steady-state PE order per iteration: mmA(b), transposes(b+2),
            # finalize(b-1), mmB(b) — transposes + finalize hide the xWt
            # evacuation latency so mmB never stalls.
            loads = {0: emit_load(0, split_queues=True)}
            emit_w()
            if bpc > 1:
                loads[1] = emit_load(1, split_queues=True)
            emit_consts()
            prev = None
            for b in range(bpc):
                x16, xT = loads.pop(b)
                xWt = emit_mmA(b, xT)
                if b + 2 < bpc:
                    loads[b + 2] = emit_load(b + 2)
                osb = out_pool.tile([NP, NC_I, H], f32, tag="osb")
                if prev is not None:
                    for ic in range(NC_I):
                        emit_finalize_ic(prev, ic)
                ET = []
                for jc in range(NC_I):
                    emit_mmB_jc(b, xT, xWt, jc, ET)
                prev = {"b": b, "x16": x16, "ET": ET, "osb": osb}
            for ic in range(NC_I):
                emit_finalize_ic(prev, ic, last=True)

    nc.compile()
    return nc


def _get_nc(bpc=BPC):
    if bpc not in _CACHE:
        _CACHE[bpc] = _build(bpc)
    return _CACHE[bpc]


def make_in_maps(arg_embeddings, relation_W, relation_b, bpc=BPC):
    x = np.ascontiguousarray(arg_embeddings, dtype=np.float32)
    W = np.ascontiguousarray(relation_W, dtype=np.float32)
    bb = np.asarray(relation_b, dtype=np.float32).reshape(1, 1)
    return [
        {
            "arg_embeddings": np.ascontiguousarray(x[c * bpc : (c + 1) * bpc]),
            "relation_W": W,
            "relation_b": bb,
        }
        for c in range(NCORES)
    ]


def kernel(arg_embeddings, relation_W, relation_b):
    from concourse.bass_utils import run_bass_kernel_spmd

    nc = _get_nc()
    in_maps = make_in_maps(arg_embeddings, relation_W, relation_b)
    res = run_bass_kernel_spmd(nc, in_maps, core_ids=list(range(NCORES)))
    out = np.concatenate([res.results[c]["out"] for c in range(NCORES)], axis=0)
    return np.ascontiguousarray(out, dtype=np.float32)


# revision 16
# speedup vs baseline: 1.0388x; 1.0247x over previous
"""Trainium2 Bass kernel: ArgumentRelationAttention.

out[b] = softmax_j(mask_diag(x[b] @ W @ x[b]^T + bias)) @ x[b]
  x: [64, 512, 768] f32, W: [768, 768] f32, bias: [1] f32

Strategy: pure batch data parallelism — 8 batches per NeuronCore x 8 cores.
Per batch everything stays on-chip:
  xT   = PE-transpose(x), f32r, 4 transposes accumulated per PSUM bank
  xWt[k,i] = sum_h W[h,k] xT[h,i]          (36 mm, f32r full-rate fp32)
  S    = (xW) @ x^T                        (24 mm, f32r)
  row softmax: S + additive diag/bias mask (DVE), then exp with a fixed
  -60 stability offset + row-sum in one ScalarE pass (output bf16) —
  softmax is shift-invariant and the score distribution (std ~15.4,
  global max ~84) keeps exp(s-60) within f32/bf16 range, so no per-row
  max reduction is needed,
  E^T  = PE-transpose(E) in bf16,
  out  = diag(1/Z) * E @ x                 (32 mm, bf16), row scale fused
         into the PSUM->SBUF evacuation.

The diagonal is excluded via an additive -30000 mask (the reference scores
the diagonal at exactly 0, whose softmax weight ~e^-40 is far below f32
noise for these score magnitudes). Batches are software-pipelined: x loads
(+ transposes) run one batch ahead, and finalize(b-1) is emitted after
scores(b), so the PE stays dense and never goes HAM-cold. The walrus
verifier requires tensors consumed by FP32r matmuls to be produced as
FP32r, so matmul-feeding tiles are declared float32r and their producing
copies/DMAs write that dtype.
"""

import numpy as np

B, N, H = 64, 512, 768
NCORES = 8
BPC = B // NCORES   # batches per core
NP = 128            # SBUF partitions
NC_I = N // NP      # 4 chunks of the sequence dim
NC_H = H // NP      # 6 chunks of the hidden dim
FH = 384            # mm-C free-dim split (768 = 2*384, <= 512 fp32 PSUM bank)
NEG_BIG = -30000.0

_CACHE = {}


def _build(bpc=BPC, mm_dtype_name="float32r"):
    import concourse.bass as bass  # noqa: F401
    import concourse.tile as tile
    from concourse import bacc, mybir
    from concourse.bass import ts, ds

    f32 = mybir.dt.float32
    bf16 = mybir.dt.bfloat16
    mdt = getattr(mybir.dt, mm_dtype_name)

    nc = bacc.Bacc(
        "TRN2",
        target_bir_lowering=False,
        debug=False,
        enable_asserts=True,
        num_devices=NCORES,
    )
    x_ext = nc.dram_tensor("arg_embeddings", [bpc, N, H], mdt, kind="ExternalInput").ap()
    w_ext = nc.dram_tensor("relation_W", [H, H], mdt, kind="ExternalInput").ap()
    b_ext = nc.dram_tensor("relation_b", [1, 1], f32, kind="ExternalInput").ap()
    out_ext = nc.dram_tensor("out", [bpc, N, H], f32, kind="ExternalOutput").ap()

    with tile.TileContext(nc) as tc:
        with (
            tc.tile_pool(name="const", bufs=1) as const_pool,
            tc.tile_pool(name="w", bufs=1) as w_pool,
            tc.tile_pool(name="xnat", bufs=4) as xnat_pool,
            tc.tile_pool(name="x16", bufs=4) as x16_pool,
            tc.tile_pool(name="xT", bufs=3 * NC_H) as xT_pool,
            tc.tile_pool(name="xWt", bufs=2 * NC_H) as xWt_pool,
            tc.tile_pool(name="ssb", bufs=3) as s_pool,
            tc.tile_pool(name="e", bufs=2 * NC_I) as e_pool,
            tc.tile_pool(name="et", bufs=2 * NC_I) as et_pool,
            tc.tile_pool(name="stat", bufs=2 * NC_I) as stat_pool,
            tc.tile_pool(name="osb", bufs=NC_I) as out_pool,
            tc.tile_pool(name="psT", bufs=2, space="PSUM") as psT_pool,
            tc.tile_pool(name="psA", bufs=2, space="PSUM") as psA_pool,
            tc.tile_pool(name="psS", bufs=2, space="PSUM") as psS_pool,
            tc.tile_pool(name="psC", bufs=2, space="PSUM") as psC_pool,
        ):
            # identity first — it gates batch 0's transposes
            ident_f32 = const_pool.tile([NP, NP], f32, tag="ident_f32")
            from concourse.masks import make_identity

            make_identity(nc, ident_f32[:])
            ident = const_pool.tile([NP, NP], mdt, tag="ident")
            nc.vector.tensor_copy(out=ident[:], in_=ident_f32[:])
            ident16 = const_pool.tile([NP, NP], bf16, tag="ident16")
            nc.vector.tensor_copy(out=ident16[:], in_=ident_f32[:])

            def emit_load(b):
                x_nat = xnat_pool.tile([NP, NC_I, H], mdt, tag="xnat")
                for ic in range(NC_I):
                    nc.sync.dma_start(x_nat[:, ic, :], x_ext[b][ts(ic, NP), :])
                x16 = x16_pool.tile([NP, NC_I, H], bf16, tag="x16")
                nc.vector.tensor_copy(out=x16[:], in_=x_nat[:])

                # x^T chunks via PE transposes, 4 per PSUM bank
                xT = []
                for hc in range(NC_H):
                    pt = psT_pool.tile([NP, N], mdt, tag="psT")
                    for ic in range(NC_I):
                        nc.tensor.matmul(
                            pt[:, ts(ic, NP)],
                            x_nat[:, ic, ts(hc, NP)],
                            ident[:],
                            is_transpose=True,
                            start=(ic == 0),
                            stop=(ic == NC_I - 1),
                        )
                    xt = xT_pool.tile([NP, N], mdt, tag="xT")
                    nc.scalar.copy(out=xt[:], in_=pt[:])
                    xT.append(xt)
                return x16, xT

            def emit_consts():
                # additive mask: NEG_BIG on the diagonal, +bias everywhere else
                masks = const_pool.tile([NP, NC_I, N], f32, tag="masks")
                nc.vector.memset(masks[:], 0.0)
                for ic in range(NC_I):
                    nc.gpsimd.affine_select(
                        out=masks[:, ic, :],
                        in_=masks[:, ic, :],
                        compare_op=mybir.AluOpType.not_equal,
                        fill=NEG_BIG,
                        base=ic * NP,
                        channel_multiplier=1,
                        pattern=[[-1, N]],
                    )
                neg60 = const_pool.tile([NP, 1], f32, tag="neg60")
                nc.vector.memset(neg60[:], -60.0)
                C["neg60"] = neg60
                b_row = const_pool.tile([1, 1], f32, tag="brow")
                nc.sync.dma_start(b_row[:], b_ext[:])
                b_col = const_pool.tile([NP, 1], f32, tag="bcol")
                nc.gpsimd.partition_broadcast(b_col[:], b_row[:])
                nc.vector.tensor_scalar_add(masks[:], masks[:], b_col[:])

                w_tile = w_pool.tile([NP, NC_H, H], mdt, tag="w")
                for hc in range(3):
                    nc.sync.dma_start(w_tile[:, hc, :], w_ext[ts(hc, NP), :])
                return masks, w_tile

            C = {}

            def emit_mmA(b, x_nat, xT):
                w_tile = C["w"]
                # xWt[kc][p, i] = sum_h W[h, kc*128+p] * x[i, h]
                xWt = []
                for kc in range(NC_H):
                    ps = psA_pool.tile([NP, N], f32, tag="psA")
                    for hc in range(NC_H):
                        nc.tensor.matmul(
                            ps[:],
                            w_tile[:, hc, ts(kc, NP)],
                            xT[hc][:],
                            start=(hc == 0),
                            stop=(hc == NC_H - 1),
                        )
                    xw = xWt_pool.tile([NP, N], mdt, tag="xWt")
                    nc.vector.tensor_copy(out=xw[:], in_=ps[:])
                    xWt.append(xw)
                return xWt

            def emit_mmB(b, x_nat, xT, xWt):
                masks = C["masks"]
                # S chunk ic: S[p, j] = sum_k xWt[k, ic*128+p] * xT[k, j]
                E, R = [], []
                for ic in range(NC_I):
                    ps = psS_pool.tile([NP, N], f32, tag="psS")
                    for kc in range(NC_H):
                        nc.tensor.matmul(
                            ps[:],
                            xWt[kc][:, ts(ic, NP)],
                            xT[kc][:],
                            start=(kc == 0),
                            stop=(kc == NC_H - 1),
                        )
                    # ssb = S + mask(bias, diag); softmax is shift-invariant
                    # so a fixed -60 stability offset replaces the row max
                    # (scores ~N(0, 15.4^2): global max ~84 -> exp(s-60)<=e^24,
                    # row max >= ~30 -> Z >= e^-30, both comfortably f32/bf16)
                    ssb = s_pool.tile([NP, N], f32, tag="ssb")
                    nc.vector.tensor_add(ssb[:], ps[:], masks[:, ic, :])
                    e = e_pool.tile([NP, N], bf16, tag="e")
                    z = stat_pool.tile([NP, 1], f32, tag="z")
                    nc.scalar.activation(
                        e[:],
                        ssb[:],
                        mybir.ActivationFunctionType.Exp,
                        bias=C["neg60"][:],
                        scale=1.0,
                        accum_out=z[:],
                    )
                    r = stat_pool.tile([NP, 1], f32, tag="r")
                    nc.vector.reciprocal(r[:], z[:])
                    E.append(e)
                    R.append(r)
                return {"x16": x_nat, "E": E, "R": R, "b": b}


            def emit_finalize(st):
                b, x16, E, R = st["b"], st["x16"], st["E"], st["R"]
                # E^T chunks (bf16) via PE transposes, 4 per PSUM bank
                ET = []
                for jc in range(NC_I):
                    pt16 = psT_pool.tile([NP, N], bf16, tag="psT")
                    for ic in range(NC_I):
                        nc.tensor.matmul(
                            pt16[:, ts(ic, NP)],
                            E[ic][:, ts(jc, NP)],
                            ident16[:],
                            is_transpose=True,
                            start=(ic == 0),
                            stop=(ic == NC_I - 1),
                        )
                    et = et_pool.tile([NP, N], bf16, tag="et")
                    nc.vector.tensor_copy(out=et[:], in_=pt16[:])
                    ET.append(et)

                # out chunk ic: out[p, h] = r[p] * sum_j E[ic*128+p, j] x[j, h]
                for ic in range(NC_I):
                    osb = out_pool.tile([NP, H], f32, tag="osb")
                    for nh in range(2):
                        ps = psC_pool.tile([NP, FH], f32, tag="psC")
                        for jc in range(NC_I):
                            nc.tensor.matmul(
                                ps[:],
                                ET[jc][:, ts(ic, NP)],
                                x16[:, jc, ds(nh * FH, FH)],
                                start=(jc == 0),
                                stop=(jc == NC_I - 1),
                            )
                        nc.scalar.activation(
                            osb[:, ds(nh * FH, FH)],
                            ps[:],
                            mybir.ActivationFunctionType.Copy,
                            scale=R[ic][:],
                        )
                    nc.sync.dma_start(out_ext[b][ts(ic, NP), :], osb[:])

            # batch 0's x load + transposes get DMA priority over W/masks.
            # Static PE order per iteration: mmA(b), transposes(b+2),
            # finalize(b-1), mmB(b) — so mmB never stalls on the xWt
            # evacuations (the transposes + finalize hide that latency).
            loads = {0: emit_load(0)}
            C["masks"], C["w"] = emit_consts()
            if bpc > 1:
                loads[1] = emit_load(1)
            for hc in range(3, NC_H):
                nc.sync.dma_start(C["w"][:, hc, :], w_ext[ts(hc, NP), :])
            prev = None
            for b in range(bpc):
                x16, xT = loads.pop(b)
                xWt = emit_mmA(b, x16, xT)
                if b + 2 < bpc:
                    loads[b + 2] = emit_load(b + 2)
                if prev is not None:
                    emit_finalize(prev)
                prev = emit_mmB(b, x16, xT, xWt)
            emit_finalize(prev)

    nc.compile()
    return nc


def _get_nc(bpc=BPC, mm_dtype_name="float32r"):
    key = (bpc, mm_dtype_name)
    if key not in _CACHE:
        _CACHE[key] = _build(bpc, mm_dtype_name)
    return _CACHE[key]


def make_in_maps(arg_embeddings, relation_W, relation_b, bpc=BPC):
    x = np.ascontiguousarray(arg_embeddings, dtype=np.float32)
    W = np.ascontiguousarray(relation_W, dtype=np.float32)
    bb = np.asarray(relation_b, dtype=np.float32).reshape(1, 1)
    return [
        {
            "arg_embeddings": np.ascontiguousarray(x[c * bpc : (c + 1) * bpc]),
            "relation_W": W,
            "relation_b": bb,
        }
        for c in range(NCORES)
    ]


def kernel(arg_embeddings, relation_W, relation_b):
    from concourse.bass_utils import run_bass_kernel_spmd

    nc = _get_nc()
    in_maps = make_in_maps(arg_embeddings, relation_W, relation_b)
    res = run_bass_kernel_spmd(nc, in_maps, core_ids=list(range(NCORES)))
    out = np.concatenate([res.results[c]["out"] for c in range(NCORES)], axis=0)
    return np.ascontiguousarray(out, dtype=np.float32)

